# revision 33
# baseline (speedup 1.0000x reference)
"""Trainium2 Bass kernel for nn_Decoder (LSTM decoder + vocab projection).

Strategy (8 NeuronCores):
- Tensor-parallel shard of the LSTM gate dimension: core m computes gate
  columns [g|i|f|o] for hidden dims [128m:128(m+1)]. The per-step h chunk
  [64,128] is transposed on the PE and AllGathered (16KB bf16/rank) into a
  shared DRAM hT buffer.
- The x-part of the gates (X @ W_ih + zemb @ W_ih + biases) is precomputed
  at full 128-row PE efficiency into DRAM, emitted inside the early
  AllGather windows; the per-step loop folds it in with one identity-matmul
  into the gate PSUM group (prefetched 2 steps ahead).
- Vocab-sharded projection (proj_W rows [4000m:4000(m+1)]) in bf16,
  scheduled so the PE crunches projection matmuls inside every AllGather
  window; PSUM drains happen inline right after each unit so banks recycle.
- Matmuls: f32r for the x/z path (exact), bf16 for W_hh and projection.
"""
import os
import sys

sys.path.insert(0, "/opt/trn_rl_repo")

import numpy as np

N_CORES = 8
VOCAB, DIM_EMB, DIM_H, DIM_Z = 32000, 512, 1024, 256
SL, BS = 64, 64
GH = DIM_H // N_CORES          # 128 hidden dims per core
GW = 4 * GH                    # 512 packed gate cols per core [g|i|f|o]
VS = VOCAB // N_CORES          # 4000 vocab rows per core
NBANK = (VS + 511) // 512      # 8 vocab banks (last is 416)
NTOK = SL * BS                 # 4096
NTILE = NTOK // 128            # 32 token tiles (= 32 xg tiles, 32 proj pairs)
KH = DIM_H // 128              # 8 h-contraction chunks
KX = DIM_EMB // 128            # 4 x-contraction chunks

_BUILT = None


def _build():
    import concourse.bacc as bacc
    import concourse.bass as bass
    import concourse.mybir as mybir
    import concourse.tile as tile
    from concourse.masks import make_identity

    f32 = mybir.dt.float32
    f32r = mybir.dt.float32r
    bf16 = mybir.dt.bfloat16
    fp8 = mybir.dt.float8e4
    DR = mybir.MatmulPerfMode.DoubleRow
    AFT = mybir.ActivationFunctionType
    use_fp8h = os.environ.get("FP8H", "1") == "1"

    nc = bacc.Bacc("TRN2", target_bir_lowering=False, debug=False,
                   num_devices=N_CORES)

    # ---- I/O ----
    XT_in = nc.dram_tensor("XT", [KX, 128, NTOK], bf16, kind="ExternalInput")
    zT_in = nc.dram_tensor("zT", [2, 128, BS], f32r, kind="ExternalInput")
    z2T_in = nc.dram_tensor("z2T", [2, 128, DIM_EMB], f32r, kind="ExternalInput")
    z2b_in = nc.dram_tensor("z2b", [1, DIM_EMB], f32r, kind="ExternalInput")
    WihT_in = nc.dram_tensor("WihT", [KX, 128, GW], f32r, kind="ExternalInput")
    WhhT_in = nc.dram_tensor("WhhT", [KH, 128, GW],
                             fp8 if use_fp8h else bf16, kind="ExternalInput")
    bg_in = nc.dram_tensor("bg", [1, GW], f32r, kind="ExternalInput")
    projWT_in = nc.dram_tensor("projWT", [KH, 128, VS], bf16, kind="ExternalInput")
    projb_in = nc.dram_tensor("projb", [128, VS], bf16, kind="ExternalInput")
    logits_out = nc.dram_tensor("logits", [SL, BS, VS], bf16,
                                kind="ExternalOutput")

    hsT_steps = nc.dram_tensor("hsT_steps", [SL, DIM_H, BS], bf16,
                               kind="Internal", addr_space="Shared")
    xg_dram = nc.dram_tensor("xg_dram", [SL, BS, GW], f32r, kind="Internal")

    rg = [list(range(N_CORES))]

    with tile.TileContext(nc) as tc:
        with (
            tc.tile_pool(name="cw", bufs=1) as cw,
            tc.tile_pool(name="sth", bufs=3) as sth,       # hr slices
            tc.tile_pool(name="stg", bufs=2) as stg,       # gate tail tiles
            tc.tile_pool(name="stx", bufs=4) as stx,       # xg prefetch stage
            tc.tile_pool(name="stp", bufs=10) as stp,      # proj out stage
            tc.tile_pool(name="sto", bufs=3) as sto,       # xg out stage
            tc.tile_pool(name="stl", bufs=5) as stl,       # proj lhsT stage
            tc.tile_pool(name="psg", bufs=2, space="PSUM") as psg,
            tc.tile_pool(name="pst", bufs=1, space="PSUM") as pst,
            tc.tile_pool(name="psp", bufs=4, space="PSUM") as psp,
            tc.tile_pool(name="drp", bufs=3, space="DRAM") as drp,
        ):
            # ---- constants & weights ----
            id64_f = cw.tile([64, 64], f32)
            make_identity(nc, id64_f[:])
            id64_r = cw.tile([64, 64], f32r)
            nc.vector.tensor_copy(id64_r[:], id64_f[:])
            # stacked identity [64, 128]: stk2[b, tok] = (tok % 64 == b)
            stk2 = cw.tile([64, 128], f32r)
            nc.vector.tensor_copy(stk2[:, 0:64], id64_f[:])
            nc.vector.tensor_copy(stk2[:, 64:128], id64_f[:])
            ones_f = cw.tile([1, 128], f32)
            nc.gpsimd.memset(ones_f[:], 1.0)
            ones_r = cw.tile([1, 128], f32r)
            nc.vector.tensor_copy(ones_r[:], ones_f[:])

            # load order matters: WihT + first XT block unblock the xg tiles
            # and step 0; WhhT unblocks step 1; projWT only needed from t>=2.
            WihT_sb = cw.tile([128, KX * GW], f32r)
            nc.scalar.dma_start(
                WihT_sb[:].rearrange("p (k g) -> p k g", k=KX),
                WihT_in.ap().rearrange("k p g -> p k g"))
            XT_sb = cw.tile([128, KX * NTOK], bf16)
            NTB = NTOK // 4
            nc.scalar.dma_start(
                XT_sb[:].rearrange("p (k t) -> p k t", k=KX)[:, :, 0:NTB],
                XT_in.ap().rearrange("k p t -> p k t")[:, :, 0:NTB])
            WihTb = cw.tile([128, KX * GW], bf16)
            nc.vector.tensor_copy(WihTb[:], WihT_sb[:])
            WhhT_sb = cw.tile([128, KH * GW], fp8 if use_fp8h else bf16)
            nc.scalar.dma_start(
                WhhT_sb[:].rearrange("p (k g) -> p k g", k=KH),
                WhhT_in.ap().rearrange("k p g -> p k g"))
            projWT_sb = cw.tile([128, KH * VS], bf16)

            def load_late_weights():
                # emitted after the pre-loop xg tiles so the first steps'
                # xg stores/prefetches aren't stuck behind ~12MB of weights
                for b in range(1, 4):
                    nc.scalar.dma_start(
                        XT_sb[:].rearrange("p (k t) -> p k t", k=KX)
                        [:, :, NTB * b:NTB * (b + 1)],
                        XT_in.ap().rearrange("k p t -> p k t")
                        [:, :, NTB * b:NTB * (b + 1)])
                nc.scalar.dma_start(
                    projWT_sb[:].rearrange("p (k v) -> p k v", k=KH),
                    projWT_in.ap().rearrange("k p v -> p k v"))
            zT_sb = cw.tile([128, 2 * BS], f32r)
            nc.sync.dma_start(
                zT_sb[:].rearrange("p (k b) -> p k b", k=2),
                zT_in.ap().rearrange("k p b -> p k b"))
            z2T_sb = cw.tile([128, 2 * DIM_EMB], f32r)
            nc.sync.dma_start(
                z2T_sb[:].rearrange("p (k e) -> p k e", k=2),
                z2T_in.ap().rearrange("k p e -> p k e"))
            z2b_sb = cw.tile([1, DIM_EMB], f32r)
            nc.sync.dma_start(z2b_sb[:], z2b_in.ap())
            bg_sb = cw.tile([1, GW], f32r)
            nc.sync.dma_start(bg_sb[:], bg_in.ap())
            projb_sb = cw.tile([128, VS], bf16)
            nc.scalar.dma_start(projb_sb[:], projb_in.ap())

            c_sb = cw.tile([64, GH], f32)

            # ---- zemb = z @ z2emb_W.T + z2emb_b ; zWihb = zemb @ Wih + bg ----
            pz = psg.tile([64, DIM_EMB], f32, tag="gpsum")
            nc.tensor.matmul(pz[:], zT_sb[:, 0:BS], z2T_sb[:, 0:DIM_EMB],
                             start=True, stop=False)
            nc.tensor.matmul(pz[:], zT_sb[:, BS:2 * BS],
                             z2T_sb[:, DIM_EMB:2 * DIM_EMB],
                             start=False, stop=False)
            nc.tensor.matmul(pz[:], ones_r[:, 0:64], z2b_sb[:],
                             start=False, stop=True)
            zemb_f = cw.tile([64, DIM_EMB], f32)
            nc.vector.tensor_copy(zemb_f[:], pz[:])
            zembT = cw.tile([128, KX * 64], f32r)
            for k in range(KX):
                pzt = pst.tile([128, 64], f32, tag="tpsum")
                nc.tensor.transpose(pzt[:], zemb_f[:, 128 * k:128 * (k + 1)],
                                    id64_f[:])
                nc.vector.tensor_copy(zembT[:, 64 * k:64 * (k + 1)], pzt[:])
            pzw = psg.tile([64, GW], f32, tag="gpsum")
            for k in range(KX):
                nc.tensor.matmul(pzw[:], zembT[:, 64 * k:64 * (k + 1)],
                                 WihT_sb[:, GW * k:GW * (k + 1)],
                                 start=(k == 0), stop=False)
            nc.tensor.matmul(pzw[:], ones_r[:, 0:64], bg_sb[:],
                             start=False, stop=True)
            zWihb_sb = cw.tile([64, GW], f32r)
            nc.vector.tensor_copy(zWihb_sb[:], pzw[:])

            # ---- xg tile emitter ----
            # xg tile i covers tokens [128i, 128(i+1)) = steps (2i, 2i+1);
            # xg[t] = X[t] @ Wih + zemb @ Wih + b_ih + b_hh, f32r in DRAM.
            xg_emitted = [0]

            def emit_xg_tile():
                i = xg_emitted[0]
                if i >= NTILE:
                    return False
                xg_emitted[0] += 1
                pp = psp.tile([128, 512], f32, name="proj_psum")
                for k in range(KX):
                    nc.tensor.matmul(
                        pp[:],
                        XT_sb[:, NTOK * k + 128 * i: NTOK * k + 128 * (i + 1)],
                        WihTb[:, GW * k:GW * (k + 1)],
                        start=(k == 0), stop=False)
                nc.tensor.matmul(pp[:], stk2[:], zWihb_sb[:],
                                 start=False, stop=True)
                ob = sto.tile([128, 512], f32r, name="xg_out")
                nc.vector.tensor_copy(ob[:], pp[:])
                nc.scalar.dma_start(xg_dram.ap()[2 * i], ob[0:64, :])
                nc.scalar.dma_start(xg_dram.ap()[2 * i + 1], ob[64:128, :])
                return True

            # ---- projection unit emitter ----
            proj_backlog = []      # (j, v) units ready to emit
            lhsT_tiles = {}        # j -> sbuf tile

            def build_lt(j, hr_a, hr_b):
                # assemble the proj lhsT pair tile from the two hr SBUF tiles
                # already loaded for the recurrence — no DRAM re-read
                lt = stl.tile([128, KH * 128], bf16, name="lhsT_stage")
                nc.vector.tensor_copy(
                    lt[:].rearrange("p (k c) -> p k c", k=KH)[:, :, 0:64],
                    hr_a[:].rearrange("p (k c) -> p k c", k=KH))
                nc.vector.tensor_copy(
                    lt[:].rearrange("p (k c) -> p k c", k=KH)[:, :, 64:128],
                    hr_b[:].rearrange("p (k c) -> p k c", k=KH))
                lhsT_tiles[j] = lt

            def emit_proj_mms(j, v):
                nv = min(512, VS - 512 * v)
                lt = lhsT_tiles[j]
                pp = psp.tile([128, 512], f32, name="proj_psum")
                for k in range(KH):
                    nc.tensor.matmul(
                        pp[:, 0:nv],
                        lt[:, 128 * k:128 * (k + 1)],
                        projWT_sb[:, VS * k + 512 * v: VS * k + 512 * v + nv],
                        start=(k == 0), stop=(k == KH - 1))
                return pp

            pending_stores = []

            def emit_proj_copy(j, v, pp):
                # PSUM->SBUF drain (DVE) happens now so the bank recycles and
                # DVE stays warm through the AllGather window; the HBM store
                # is deferred so the collective runs on quiet SDMA engines.
                nv = min(512, VS - 512 * v)
                ob = stp.tile([128, 512], bf16, name="proj_out")
                nc.vector.tensor_tensor(ob[:, 0:nv], pp[:, 0:nv],
                                        projb_sb[:, 512 * v:512 * v + nv],
                                        op=mybir.AluOpType.add)
                pending_stores.append((j, v, ob))
                if v == NBANK - 1:
                    del lhsT_tiles[j]

            def flush_stores():
                while pending_stores:
                    j, v, ob = pending_stores.pop(0)
                    nv = min(512, VS - 512 * v)
                    nc.scalar.dma_start(
                        logits_out.ap()[2 * j, :, 512 * v:512 * v + nv],
                        ob[0:64, 0:nv])
                    nc.scalar.dma_start(
                        logits_out.ap()[2 * j + 1, :, 512 * v:512 * v + nv],
                        ob[64:128, 0:nv])

            def pump(budget):
                done = 0
                while done < budget:
                    if proj_backlog:
                        j, v = proj_backlog.pop(0)
                        pp = emit_proj_mms(j, v)
                        emit_proj_copy(j, v, pp)
                    elif not emit_xg_tile():
                        break
                    done += 1
                return done

            # ---- xg prefetch staging ----
            xgt_tiles = {}

            def prefetch_xg(t):
                if t >= SL:
                    return
                xt = stx.tile([64, GW], f32r, name="xg_stage")
                nc.sync.dma_start(xt[:], xg_dram.ap()[t])
                xgt_tiles[t] = xt

            # ---- pre-loop: first xg tiles + prefetch ----
            emit_xg_tile()
            emit_xg_tile()
            prefetch_xg(0)
            prefetch_xg(1)
            prefetch_xg(2)
            load_late_weights()

            n_steps = int(os.environ.get("KSTEPS", str(SL)))
            post_units = int(os.environ.get("POST_UNITS", "3"))
            valve = int(os.environ.get("VALVE", "10"))

            # step 0 gate psum: x-part only
            pg = psg.tile([64, GW], f32, name="gate_psum", tag="gpsum")
            nc.tensor.matmul(pg[:], id64_r[:], xgt_tiles.pop(0)[:],
                             start=True, stop=True)

            hr_prev = None
            for t in range(n_steps):
                hr = None
                if t > 0:
                    # consumer DMA of AllGather(t-1) output, then h-part
                    hr = sth.tile([128, KH * 64], bf16, name="hr")
                    nc.sync.dma_start(
                        hr[:].rearrange("p (k c) -> p k c", k=KH),
                        hsT_steps.ap()[t - 1].rearrange("(k p) c -> p k c",
                                                        k=KH))
                    if use_fp8h:
                        hr8 = sth.tile([128, KH * 64], fp8, name="hr8")
                        nc.vector.tensor_copy(hr8[:], hr[:])
                        for q in range(KH // 2):
                            nc.tensor.matmul(
                                pg[:],
                                hr8[:, 128 * q:128 * (q + 1)].rearrange(
                                    "p (two c) -> p two c", two=2),
                                WhhT_sb[:, 2 * GW * q:2 * GW * (q + 1)]
                                .rearrange("p (two g) -> p two g", two=2),
                                perf_mode=DR,
                                start=False, stop=(q == KH // 2 - 1))
                    else:
                        for k in range(KH):
                            nc.tensor.matmul(pg[:], hr[:, 64 * k:64 * (k + 1)],
                                             WhhT_sb[:, GW * k:GW * (k + 1)],
                                             start=False, stop=(k == KH - 1))
                    if t % 2 == 0:
                        # hr_prev holds hsT[t-2], hr holds hsT[t-1]
                        j = t // 2 - 1
                        build_lt(j, hr_prev, hr)
                        proj_backlog.extend((j, v) for v in range(NBANK))
                flush_stores()

                # ---- gate tail (ACT/DVE) ----
                # ACT order: tanh(g), sigmoid(i,f) feed the c-chain first;
                # sigmoid(o) runs while the DVE c-chain executes.
                act = stg.tile([64, GW], f32, name="act_tile")
                nc.scalar.activation(act[:, 0:GH], pg[:, 0:GH], AFT.Tanh)
                nc.scalar.activation(act[:, GH:3 * GH], pg[:, GH:3 * GH],
                                     AFT.Sigmoid)
                tmp = stg.tile([64, GH], f32, name="tmp_ig")
                nc.vector.tensor_mul(tmp[:], act[:, GH:2 * GH], act[:, 0:GH])
                if t > 0:
                    nc.vector.tensor_mul(c_sb[:], act[:, 2 * GH:3 * GH], c_sb[:])
                    nc.vector.tensor_add(c_sb[:], c_sb[:], tmp[:])
                else:
                    nc.vector.tensor_copy(c_sb[:], tmp[:])
                nc.scalar.activation(act[:, 3 * GH:4 * GH], pg[:, 3 * GH:4 * GH],
                                     AFT.Sigmoid)
                tct = stg.tile([64, GH], f32, name="tanh_c")
                nc.scalar.activation(tct[:], c_sb[:], AFT.Tanh)
                h_sb = stg.tile([64, GH], f32, name="h_tile")
                nc.vector.tensor_mul(h_sb[:], act[:, 3 * GH:4 * GH], tct[:])

                # ---- pre-slot PE work (runs while ACT/DVE tail computes) ----
                if t + 1 < n_steps:
                    pg = psg.tile([64, GW], f32, name="gate_psum", tag="gpsum")
                    nc.tensor.matmul(pg[:], id64_r[:],
                                     xgt_tiles.pop(t + 1)[:],
                                     start=True, stop=False)
                pre_pp = None
                need_xg = xg_emitted[0] < min(NTILE, t // 2 + 4)
                if need_xg:
                    emit_xg_tile()
                elif proj_backlog:
                    pj, pv = proj_backlog.pop(0)
                    pre_pp = (pj, pv, emit_proj_mms(pj, pv))
                else:
                    emit_xg_tile()

                # ---- transpose h, stage, AllGather ----
                pt = pst.tile([128, 64], f32, name="hT_psum", tag="tpsum")
                nc.tensor.transpose(pt[:], h_sb[:], id64_f[:])
                ht = stg.tile([128, 64], bf16, name="hT_stage")
                nc.vector.tensor_copy(ht[:], pt[:])
                cc_in = drp.tile([128, 64], bf16, name="cc_in")
                nc.sync.dma_start(cc_in[:], ht[:])
                nc.gpsimd.collective_compute(
                    "AllGather", mybir.AluOpType.bypass,
                    replica_groups=rg,
                    ins=[cc_in[:].opt()],
                    outs=[hsT_steps.ap()[t].opt()],
                )

                # ---- post-AG-trigger: fill the AllGather window ----
                if pre_pp is not None:
                    emit_proj_copy(*pre_pp)
                prefetch_xg(t + 3)
                if t < 12:
                    budget = 2          # build a standing backlog early
                elif t >= SL - 16:
                    budget = post_units + (1 if len(proj_backlog) > 3 else 0)
                else:
                    budget = post_units + (1 if len(proj_backlog) > 8 else 0)
                pump(budget)
                hr_prev = hr

            # ---- epilogue: last pair + remaining projection units ----
            if n_steps == SL:
                hr_last = sth.tile([128, KH * 64], bf16, name="hr")
                nc.sync.dma_start(
                    hr_last[:].rearrange("p (k c) -> p k c", k=KH),
                    hsT_steps.ap()[SL - 1].rearrange("(k p) c -> p k c",
                                                     k=KH))
                build_lt(SL // 2 - 1, hr_prev, hr_last)
                proj_backlog.extend((SL // 2 - 1, v) for v in range(NBANK))
            while proj_backlog:
                pump(4)
                flush_stores()
            flush_stores()

    nc.compile()
    return nc


def _prep_inputs(inputs):
    """Host-side sharding & layout. Returns per-core in_maps."""
    f32 = np.float32
    z_c = np.asarray(inputs["z_c"], f32)
    z_f = np.asarray(inputs["z_f"], f32)
    input_ids = np.asarray(inputs["input_ids"]).astype(np.int64)
    target = np.asarray(inputs["target"]).astype(np.int64)
    embed_W = np.asarray(inputs["embed_W"], f32)
    z2emb_W = np.asarray(inputs["z2emb_W"], f32)
    z2emb_b = np.asarray(inputs["z2emb_b"], f32)
    W_ih = np.asarray(inputs["W_ih"], f32)
    W_hh = np.asarray(inputs["W_hh"], f32)
    b_ih = np.asarray(inputs["b_ih"], f32)
    b_hh = np.asarray(inputs["b_hh"], f32)
    proj_W = np.asarray(inputs["proj_W"], f32)
    proj_b = np.asarray(inputs["proj_b"], f32)

    # tokens: step 0 uses input_ids[0], step t>0 uses target[t-1]
    tokens = np.concatenate([input_ids[:1], target[:-1]], axis=0)  # [SL, BS]
    import ml_dtypes as _mld
    X = embed_W[tokens.reshape(-1)]                 # [NTOK, DIM_EMB]
    XT = np.ascontiguousarray(X.T).reshape(KX, 128, NTOK).astype(
        _mld.bfloat16)

    z = np.concatenate([z_f, z_c], axis=1)          # [BS, DIM_Z]
    zT = np.ascontiguousarray(z.T).reshape(2, 128, BS)
    z2T = np.ascontiguousarray(z2emb_W.T).reshape(2, 128, DIM_EMB)
    z2b = z2emb_b.reshape(1, DIM_EMB)
    bsum = b_ih + b_hh

    import ml_dtypes
    in_maps = []
    for m in range(N_CORES):
        sel = np.r_[2048 + GH * m:2048 + GH * (m + 1),      # g
                    0 + GH * m:0 + GH * (m + 1),            # i
                    1024 + GH * m:1024 + GH * (m + 1),      # f
                    3072 + GH * m:3072 + GH * (m + 1)]      # o
        WihT_m = np.ascontiguousarray(W_ih[sel, :].T).reshape(KX, 128, GW)
        WhhT_m = np.ascontiguousarray(W_hh[sel, :].T).reshape(
            KH, 128, GW).astype(
                ml_dtypes.float8_e4m3
                if os.environ.get("FP8H", "1") == "1"
                else ml_dtypes.bfloat16)
        bg_m = bsum[sel].reshape(1, GW)
        projWT_m = np.ascontiguousarray(
            proj_W[VS * m:VS * (m + 1), :].T).reshape(KH, 128, VS).astype(
                ml_dtypes.bfloat16)
        projb_m = np.ascontiguousarray(np.broadcast_to(
            proj_b[VS * m:VS * (m + 1)].reshape(1, VS), (128, VS))).astype(
            ml_dtypes.bfloat16)
        in_maps.append({
            "XT": XT, "zT": zT, "z2T": z2T, "z2b": z2b,
            "WihT": WihT_m, "WhhT": WhhT_m, "bg": bg_m,
            "projWT": projWT_m, "projb": projb_m,
        })
    return in_maps


def run(inputs, trace=False):
    """Run the kernel; returns (logits [SL, BS, VOCAB] f32, BassKernelResults)."""
    global _BUILT
    if _BUILT is None:
        _BUILT = _build()
    from concourse.bass_utils import run_bass_kernel_spmd
    in_maps = _prep_inputs(inputs)
    res = run_bass_kernel_spmd(_BUILT, in_maps, core_ids=list(range(N_CORES)),
                               trace=trace)
    logits = np.concatenate(
        [np.asarray(res.results[m]["logits"]).astype(np.float32)
         for m in range(N_CORES)], axis=2)
    return logits, res


def kernel(**inputs) -> np.ndarray:
    logits, _ = run(inputs, trace=False)
    return logits
